# revision 12
# baseline (speedup 1.0000x reference)
"""AFNO (Adaptive Fourier Neural Operator) Trainium2 kernel, 8-core data-parallel.

Per core: one batch element.
  spectral path: rfft2 via packed-real W-DFT + stacked cos/sin H-DFT matmuls,
  block-diag complex MLP (relu, softshrink), irfft2 via inverse DFT matmuls.
  bias path: dense 768x768 per-token linear (Conv1d k=1), staged via DRAM.
Layout rotations use the DMA xbar transpose engine in its native form:
  in [P, k*128] -> out [128, k, P]   (each [P,128] column tile transposed)
with zero-padded 128-minors (pads persist in once-allocated tiles).
All matmul inputs bf16, fp32 PSUM accumulation.
"""

import sys, os
for p in ("/opt/trn_rl_repo", "/root/.axon_site/_ro/trn_rl_repo"):
    if os.path.isdir(p) and p not in sys.path:
        sys.path.insert(0, p)

import numpy as np
import ml_dtypes
from contextlib import ExitStack

import concourse.bass as bass
from concourse import bacc
import concourse.mybir as mybir
import concourse.tile as tile
from concourse.bass import ts
from concourse.bass_utils import run_bass_kernel_spmd

BF16 = mybir.dt.bfloat16
F32 = mybir.dt.float32
NPBF16 = ml_dtypes.bfloat16
AF = mybir.ActivationFunctionType

DIM, H, W, NB = 768, 64, 64, 8
BS = DIM // NB     # 96
LAMBD = 0.01
NCORES = 8
CHUNKS = [(0, 8), (8, 8), (16, 8), (24, 8), (32, 1)]


def _build_consts():
    w = np.arange(W)
    h = np.arange(H)
    rwt = np.zeros((W, W), np.float64)          # lhsT [w, w~] fwd packed W-DFT
    for j in range(33):
        rwt[:, j] = np.cos(2 * np.pi * w * j / 64)
    for t in range(31):
        rwt[:, 33 + t] = -np.sin(2 * np.pi * w * (t + 1) / 64)
    rwt /= 64.0                                 # fwd ortho norm

    ang = 2 * np.pi * np.outer(h, h) / 64
    fha, fhb = np.cos(ang), np.sin(ang)         # A = Ch@P, B = Sh@P
    chi, shi = np.cos(ang), np.sin(ang)         # inverse-H (ifft along h')
    z = np.zeros((H, H))
    chishi = np.concatenate([chi, shi], axis=1)
    nshichi = np.concatenate([-shi, chi], axis=1)
    chishin = np.concatenate([z, chi], axis=1)      # nyq re' -> partitions 64+
    nshichin = np.concatenate([z, -shi], axis=1)

    rwinv = np.zeros((W, W), np.float64)        # [packed64, w] inverse-W
    rwinv[0, :] = 1.0
    for kk in range(1, 32):
        rwinv[kk, :] = 2 * np.cos(2 * np.pi * w * kk / 64)
    for t in range(31):                         # im w'=t+1 at row 32+t
        rwinv[32 + t, :] = -2 * np.sin(2 * np.pi * w * (t + 1) / 64)
    rwinv[63, :] = np.cos(np.pi * w)            # nyq-re at row 63
    rwinv /= 64.0                               # inv ortho norm

    return {n: a.astype(np.float32).astype(NPBF16) for n, a in
            [("rwt", rwt), ("fha", fha), ("fhb", fhb), ("chishi", chishi),
             ("nshichi", nshichi), ("chishin", chishin),
             ("nshichin", nshichin),
             ("rwinv_re", rwinv[0:32]), ("rwinv_im", rwinv[32:64])]}


def build_nc():
    nc = bacc.Bacc("TRN2", target_bir_lowering=False, debug=False)

    xs_d = nc.declare_dram_parameter("xs", [NB, W, BS, H], BF16, isOutput=False)
    xt_d = nc.declare_dram_parameter("xt", [6, 128, 4096], BF16, isOutput=False)
    out_d = nc.declare_dram_parameter("out", [W, H, DIM], F32, isOutput=True)

    cds = {}
    for n in ["rwt", "fha", "fhb"]:
        cds[n] = nc.declare_dram_parameter(n, [64, 64], BF16, isOutput=False)
    for n in ["rwinv_re", "rwinv_im"]:
        cds[n] = nc.declare_dram_parameter(n, [32, 64], BF16, isOutput=False)
    for n in ["chishi", "nshichi", "chishin", "nshichin"]:
        cds[n] = nc.declare_dram_parameter(n, [64, 128], BF16, isOutput=False)
    for n in ["w10", "w11", "w11n", "w20", "w21", "w21n"]:
        cds[n] = nc.declare_dram_parameter(n, [BS, NB * BS], BF16, isOutput=False)
    for n in ["b1r", "b1i", "a1r", "a2r", "a1i", "a2i", "bbp"]:
        cds[n] = nc.declare_dram_parameter(n, [BS, NB], F32, isOutput=False)
    wbt_d = nc.declare_dram_parameter("wbt", [6, 128, NB * BS], BF16, isOutput=False)

    with ExitStack() as ctx:
        tc = ctx.enter_context(tile.TileContext(nc))

        cpool = ctx.enter_context(tc.tile_pool(name="consts", bufs=1))
        big = ctx.enter_context(tc.tile_pool(name="big", bufs=1))
        sp = ctx.enter_context(tc.tile_pool(name="spec", bufs=1))
        sm = ctx.enter_context(tc.tile_pool(name="small", bufs=2))
        sm1 = ctx.enter_context(tc.tile_pool(name="small1", bufs=1))
        op = ctx.enter_context(tc.tile_pool(name="outp", bufs=1))
        pp = ctx.enter_context(tc.tile_pool(name="ps", bufs=5, space="PSUM"))
        ppb = ctx.enter_context(tc.tile_pool(name="psb", bufs=2, space="PSUM"))
        dram = ctx.enter_context(tc.tile_pool(name="dram", bufs=1, space="DRAM"))
        bias_d = dram.tile([W, H, DIM], BF16, tag="bias_scratch",
                           name="bias_scratch")

        ct = {}
        for n in ["rwt", "fha", "fhb"]:
            ct[n] = cpool.tile([64, 64], BF16, tag=f"c_{n}", name=f"c_{n}")
            nc.sync.dma_start(ct[n][:], cds[n][:])
        ct["rwinv_re"] = cpool.tile([32, 64], BF16, tag="c_rwinv_re",
                                    name="c_rwinv_re")
        nc.sync.dma_start(ct["rwinv_re"][:], cds["rwinv_re"][:])
        _rwim = cpool.tile([64, 64], BF16, tag="c_rwinv_im", name="c_rwinv_im")
        nc.sync.dma_start(_rwim[32:64, :], cds["rwinv_im"][:])
        ct["rwinv_im"] = _rwim[32:64, :]
        for n in ["chishi", "nshichi", "chishin", "nshichin"]:
            ct[n] = cpool.tile([64, 128], BF16, tag=f"c_{n}", name=f"c_{n}")
            nc.sync.dma_start(ct[n][:], cds[n][:])
        for n in ["w10", "w11", "w11n", "w20", "w21", "w21n"]:
            ct[n] = cpool.tile([BS, NB * BS], BF16, tag=f"c_{n}", name=f"c_{n}")
            nc.sync.dma_start(ct[n][:], cds[n][:])
        for n in ["b1r", "b1i", "a1r", "a2r", "a1i", "a2i", "bbp"]:
            ct[n] = cpool.tile([BS, NB], F32, tag=f"c_{n}", name=f"c_{n}")
            nc.sync.dma_start(ct[n][:], cds[n][:])
        wbt_t = []
        for kk in range(6):
            t = cpool.tile([128, NB * BS], BF16, tag=f"c_wbt{kk}",
                           name=f"c_wbt{kk}")
            nc.sync.dma_start(t[:], wbt_d[kk])
            wbt_t.append(t)

        # 12KB-class tags; bias phase borrows some for xt chunks
        BIGTAGS = ["xsj", "p1t", "s2a", "s2b", "t6a", "t6b"]

        def ev(i):
            return nc.vector.tensor_copy if (i % 2 == 0) else nc.scalar.copy

        # ========== phase A: bias path -> DRAM scratch ==========
        xtt = []
        for kk in range(6):
            t = big.tile([128, 4096], BF16, tag=BIGTAGS[kk], name=f"xt{kk}")
            nc.sync.dma_start(t[:], xt_d[kk])
            xtt.append(t)
        for j in range(NB):
            for t in range(8):
                pb = ppb.tile([BS, 512], F32, tag="psb")
                for kk in range(6):
                    nc.tensor.matmul(pb[:], wbt_t[kk][:, ts(j, BS)],
                                     xtt[kk][:, ts(t, 512)],
                                     start=(kk == 0), stop=(kk == 5))
                bx = sm.tile([BS, 512], BF16, tag="bx")
                nc.scalar.activation(bx[:], pb[:], AF.Identity,
                                     bias=ct["bbp"][:, j:j + 1])
                # native xbar: [96, 4*128] -> [128=(h2,w), 4, 96]
                bxt = sm.tile([128, 4, BS], BF16, tag="bxt")
                nc.sync.dma_start_transpose(bxt[:], bx[:])
                nc.sync.dma_start(
                    bias_d[:, 8 * t:8 * t + 8:2, ts(j, BS)], bxt[0:64])
                nc.sync.dma_start(
                    bias_d[:, 8 * t + 1:8 * t + 8:2, ts(j, BS)], bxt[64:128])

        # ========== phase B: spectral path per channel block ==========
        # persistent padded tiles (pads zeroed once)
        p1 = big.tile([W, BS, 128], BF16, tag="p1", name="p1g")
        mixr = sp.tile([H, 33, 128], BF16, tag="mixr", name="mixrg")
        mixi = sp.tile([H, 33, 128], BF16, tag="mixi", name="mixig")
        s5r = sp.tile([BS, 33, 128], BF16, tag="s5r", name="s5rg")
        s5i = sp.tile([BS, 33, 128], BF16, tag="s5i", name="s5ig")
        s6buf = sp.tile([128, BS, 128], BF16, tag="s6buf", name="s6bufg")
        nc.gpsimd.memset(p1[:, :, 64:128], 0.0)
        nc.gpsimd.memset(mixr[:, :, 96:128], 0.0)
        nc.gpsimd.memset(mixi[:, :, 96:128], 0.0)
        nc.gpsimd.memset(s5r[:, :, 64:128], 0.0)
        nc.gpsimd.memset(s5i[:, :, 64:128], 0.0)
        nc.gpsimd.memset(s6buf[0:64, :, 32:128], 0.0)
        nc.gpsimd.memset(s6buf[64:128, :, 0:32], 0.0)
        nc.gpsimd.memset(s6buf[64:128, :, 64:128], 0.0)

        for j in range(NB):
            # S1: packed-real W-DFT -> p1 [w~, (c, hpad128)]
            xsj = big.tile([W, BS, H], BF16, tag="xsj")
            nc.sync.dma_start(xsj[:], xs_d[j])
            xsf = xsj.rearrange("p a b -> p (a b)")
            for t in range(12):
                psx = pp.tile([64, 512], F32, tag="ps")
                nc.tensor.matmul(psx[:], ct["rwt"][:], xsf[:, ts(t, 512)],
                                 start=True, stop=True)
                ev(t)(p1[:, 8 * t:8 * t + 8, 0:64],
                      psx.rearrange("p (a b) -> p a b", a=8))

            # R1 native xbar: [w~, (c x 128)] -> p1t [128=(h|pad), c, w~]
            p1t = big.tile([128, BS, W], BF16, tag="p1t")
            nc.sync.dma_start_transpose(p1t[:], p1.rearrange("p a b -> p (a b)"))

            # S2: H-DFT (A = Ch@P, B = Sh@P) -> [h', (c, w~)]
            s2a = big.tile([H, BS, W], BF16, tag="s2a")
            s2b = big.tile([H, BS, W], BF16, tag="s2b")
            p1tf = p1t[0:64].rearrange("p a b -> p (a b)")
            s2af = s2a.rearrange("p a b -> p (a b)")
            s2bf = s2b.rearrange("p a b -> p (a b)")
            for t in range(12):
                psa = pp.tile([64, 512], F32, tag="ps")
                psb2 = pp.tile([64, 512], F32, tag="ps")
                nc.tensor.matmul(psa[:], ct["fha"][:], p1tf[:, ts(t, 512)],
                                 start=True, stop=True)
                nc.tensor.matmul(psb2[:], ct["fhb"][:], p1tf[:, ts(t, 512)],
                                 start=True, stop=True)
                ev(t)(s2af[:, ts(t, 512)], psa[:])
                ev(t + 1)(s2bf[:, ts(t, 512)], psb2[:])

            # MIX: A/B -> re/im  [h', (w' 33, cpad128)]
            nc.vector.tensor_add(
                mixr[:, 1:32, 0:96],
                s2a[:, :, 1:32].rearrange("p c w -> p w c"),
                s2b[:, :, 33:64].rearrange("p c w -> p w c"))
            nc.gpsimd.tensor_sub(
                mixi[:, 1:32, 0:96],
                s2a[:, :, 33:64].rearrange("p c w -> p w c"),
                s2b[:, :, 1:32].rearrange("p c w -> p w c"))
            nc.vector.tensor_copy(mixr[:, 0, 0:96], s2a[:, :, 0])
            nc.vector.tensor_copy(mixr[:, 32, 0:96], s2a[:, :, 32])
            nc.scalar.mul(mixi[:, 0, 0:96], s2b[:, :, 0], -1.0)
            nc.scalar.mul(mixi[:, 32, 0:96], s2b[:, :, 32], -1.0)

            # rot native: [h', (33 x 128)] -> [128=(c|pad), 33, h']
            mlpr = sp.tile([128, 33, H], BF16, tag="mlpr")
            mlpi = sp.tile([128, 33, H], BF16, tag="mlpi")
            nc.sync.dma_start_transpose(
                mlpr[:], mixr.rearrange("p a b -> p (a b)"))
            nc.sync.dma_start_transpose(
                mlpi[:], mixi.rearrange("p a b -> p (a b)"))

            # MLP layer 1
            r1b = sp.tile([BS, 33, H], BF16, tag="r1b")
            i1b = sp.tile([BS, 33, H], BF16, tag="i1b")
            for (w0, wc) in CHUNKS:
                n = wc * H
                rr = mlpr[0:96, w0:w0 + wc, :]
                ri = mlpi[0:96, w0:w0 + wc, :]
                pr = pp.tile([BS, 512], F32, tag="ps")
                pi = pp.tile([BS, 512], F32, tag="ps")
                nc.tensor.matmul(pr[:, :n], ct["w10"][:, ts(j, BS)], rr,
                                 start=True, stop=False)
                nc.tensor.matmul(pr[:, :n], ct["w11n"][:, ts(j, BS)], ri,
                                 start=False, stop=True)
                nc.tensor.matmul(pi[:, :n], ct["w11"][:, ts(j, BS)], rr,
                                 start=True, stop=False)
                nc.tensor.matmul(pi[:, :n], ct["w10"][:, ts(j, BS)], ri,
                                 start=False, stop=True)
                nc.scalar.activation(r1b[:, w0:w0 + wc, :], pr[:, :n],
                                     AF.Relu, bias=ct["b1r"][:, j:j + 1])
                nc.scalar.activation(i1b[:, w0:w0 + wc, :], pi[:, :n],
                                     AF.Relu, bias=ct["b1i"][:, j:j + 1])

            # MLP layer 2 + softshrink -> s5r/s5i [c, (w' 33, hpad128)]
            for (w0, wc) in CHUNKS:
                n = wc * H
                rr, ri = r1b[:, w0:w0 + wc, :], i1b[:, w0:w0 + wc, :]
                pr = pp.tile([BS, 512], F32, tag="ps")
                pi = pp.tile([BS, 512], F32, tag="ps")
                nc.tensor.matmul(pr[:, :n], ct["w20"][:, ts(j, BS)], rr,
                                 start=True, stop=False)
                nc.tensor.matmul(pr[:, :n], ct["w21n"][:, ts(j, BS)], ri,
                                 start=False, stop=True)
                nc.tensor.matmul(pi[:, :n], ct["w21"][:, ts(j, BS)], rr,
                                 start=True, stop=False)
                nc.tensor.matmul(pi[:, :n], ct["w20"][:, ts(j, BS)], ri,
                                 start=False, stop=True)
                sar = sm1.tile([BS, wc, H], BF16, tag="sar")
                sbr = sm1.tile([BS, wc, H], BF16, tag="sbr")
                sai = sm1.tile([BS, wc, H], BF16, tag="sai")
                sbi = sm1.tile([BS, wc, H], BF16, tag="sbi")
                nc.scalar.activation(sar[:], pr[:, :n], AF.Relu,
                                     bias=ct["a1r"][:, j:j + 1])
                nc.scalar.activation(sbr[:], pr[:, :n], AF.Relu,
                                     bias=ct["a2r"][:, j:j + 1], scale=-1.0)
                nc.scalar.activation(sai[:], pi[:, :n], AF.Relu,
                                     bias=ct["a1i"][:, j:j + 1])
                nc.scalar.activation(sbi[:], pi[:, :n], AF.Relu,
                                     bias=ct["a2i"][:, j:j + 1], scale=-1.0)
                nc.vector.tensor_sub(s5r[:, w0:w0 + wc, 0:64], sar[:], sbr[:])
                nc.gpsimd.tensor_sub(s5i[:, w0:w0 + wc, 0:64], sai[:], sbi[:])

            # rot native: [c, (33 x 128)] -> [128=(h'|pad), 33, c]
            t5r = sp.tile([128, 33, BS], BF16, tag="mlpr")
            t5i = sp.tile([128, 33, BS], BF16, tag="mlpi")
            nc.sync.dma_start_transpose(
                t5r[:], s5r.rearrange("p a b -> p (a b)"))
            nc.sync.dma_start_transpose(
                t5i[:], s5i.rearrange("p a b -> p (a b)"))

            # S4 inverse-H -> s6buf [128=(reh|imh), (c, minor128)]
            for ci in range(8):
                w0 = ci * 4
                rr = t5r[0:64, w0:w0 + 4, :]
                ri = t5i[0:64, w0:w0 + 4, :]
                p4 = pp.tile([128, 384], F32, tag="ps")
                nc.tensor.matmul(p4[:], ct["chishi"][:], rr,
                                 start=True, stop=False)
                nc.tensor.matmul(p4[:], ct["nshichi"][:], ri,
                                 start=False, stop=True)
                p43 = p4.rearrange("p (a b) -> p a b", a=4)
                # re half -> minor slots w0..w0+3 ; im half -> 32 + (w'-1)
                ev(ci)(s6buf[0:64, :, w0:w0 + 4],
                       p43[0:64].rearrange("p a b -> p b a"))
                if w0 == 0:   # im w'=0 (DC) discarded by irfft
                    ev(ci + 1)(s6buf[64:128, :, 32:35],
                               p43[64:128, 1:4].rearrange("p a b -> p b a"))
                else:
                    ev(ci + 1)(s6buf[64:128, :, 32 + w0 - 1:32 + w0 + 3],
                               p43[64:128].rearrange("p a b -> p b a"))
            p4n = pp.tile([128, BS], F32, tag="ps")
            nc.tensor.matmul(p4n[:], ct["chishin"][:], t5r[0:64, 32, :],
                             start=True, stop=False)
            nc.tensor.matmul(p4n[:], ct["nshichin"][:], t5i[0:64, 32, :],
                             start=False, stop=True)
            nc.vector.tensor_copy(s6buf[64:128, :, 63], p4n[64:128, :])

            # xbar native: re -> t6a rows 0..31, im+nyq -> t6b rows 32..63
            t6a = big.tile([128, BS, 64], BF16, tag="t6a")
            t6b = big.tile([128, BS, 64], BF16, tag="t6b")
            nc.sync.dma_start_transpose(
                t6a[:], s6buf[0:64].rearrange("p a b -> p (a b)"))
            nc.sync.dma_start_transpose(
                t6b[:], s6buf[64:128].rearrange("p a b -> p (a b)"))

            # S5 inverse-W (two K=32 matmuls) -> spatial [w, (c, h)]
            spo = big.tile([W, BS, H], BF16, tag="s2b", name=f"spo{j}")
            spof = spo.rearrange("p a b -> p (a b)")
            t6af = t6a[0:32].rearrange("p a b -> p (a b)")
            t6bf = t6b[32:64].rearrange("p a b -> p (a b)")
            for t in range(12):
                p5 = pp.tile([64, 512], F32, tag="ps")
                nc.tensor.matmul(p5[:], ct["rwinv_re"][:], t6af[:, ts(t, 512)],
                                 start=True, stop=False)
                nc.tensor.matmul(p5[:], ct["rwinv_im"], t6bf[:, ts(t, 512)],
                                 start=False, stop=True)
                ev(t)(spof[:, ts(t, 512)], p5[:])

            # final add with bias tiles from DRAM
            for t in range(8):
                bt = sm.tile([W, 8, BS], BF16, tag="bt")
                nc.sync.dma_start(bt[:], bias_d[:, ts(t, 8), ts(j, BS)])
                oadd = op.tile([W, 8, BS], F32, tag="oadd")
                addop = nc.vector.tensor_add if (t % 2 == 0) else \
                    nc.gpsimd.tensor_add
                addop(oadd[:], bt[:],
                      spo[:, :, ts(t, 8)].rearrange("p c h -> p h c"))
                nc.sync.dma_start(out_d[:, ts(t, 8), ts(j, BS)], oadd[:])

    return nc


_nc_cache = None


def _get_nc():
    global _nc_cache
    if _nc_cache is None:
        _nc_cache = build_nc()
        _nc_cache.finalize()
    return _nc_cache


def make_in_maps(x, w1, b1, w2, b2, Wb, bb):
    shared = dict(_build_consts())
    shared["w10"] = np.concatenate([w1[0][b] for b in range(NB)], 1).astype(NPBF16)
    shared["w11"] = np.concatenate([w1[1][b] for b in range(NB)], 1).astype(NPBF16)
    shared["w11n"] = np.concatenate([-w1[1][b] for b in range(NB)], 1).astype(NPBF16)
    shared["w20"] = np.concatenate([w2[0][b] for b in range(NB)], 1).astype(NPBF16)
    shared["w21"] = np.concatenate([w2[1][b] for b in range(NB)], 1).astype(NPBF16)
    shared["w21n"] = np.concatenate([-w2[1][b] for b in range(NB)], 1).astype(NPBF16)
    shared["b1r"] = np.ascontiguousarray(b1[0].T).astype(np.float32)
    shared["b1i"] = np.ascontiguousarray(b1[1].T).astype(np.float32)
    shared["a1r"] = np.ascontiguousarray((b2[0] - LAMBD).T).astype(np.float32)
    shared["a2r"] = np.ascontiguousarray((-b2[0] - LAMBD).T).astype(np.float32)
    shared["a1i"] = np.ascontiguousarray((b2[1] - LAMBD).T).astype(np.float32)
    shared["a2i"] = np.ascontiguousarray((-b2[1] - LAMBD).T).astype(np.float32)
    shared["bbp"] = np.ascontiguousarray(bb.reshape(NB, BS).T).astype(np.float32)
    shared["wbt"] = np.ascontiguousarray(Wb.T.reshape(6, 128, DIM)).astype(NPBF16)

    in_maps = []
    for b in range(NCORES):
        m = dict(shared)
        xg = x[b].reshape(H, W, DIM)
        m["xs"] = np.ascontiguousarray(
            xg.reshape(H, W, NB, BS).transpose(2, 1, 3, 0)).astype(NPBF16)
        m["xt"] = np.ascontiguousarray(x[b].T.reshape(6, 128, 4096)).astype(NPBF16)
        in_maps.append(m)
    return in_maps


def kernel(x, w1, b1, w2, b2, Wb, bb, _trace=False):
    nc = _get_nc()
    in_maps = make_in_maps(np.asarray(x, np.float32), np.asarray(w1, np.float32),
                           np.asarray(b1, np.float32), np.asarray(w2, np.float32),
                           np.asarray(b2, np.float32), np.asarray(Wb, np.float32),
                           np.asarray(bb, np.float32))
    res = run_bass_kernel_spmd(nc, in_maps, list(range(NCORES)), trace=_trace)
    outs = []
    for b in range(NCORES):
        o = np.asarray(res.results[b]["out"], np.float32)   # [w, h, c]
        outs.append(o.transpose(1, 0, 2).reshape(H * W, DIM))
    full = np.stack(outs, axis=0)
    if _trace:
        return full, res
    return full


# revision 14
# speedup vs baseline: 1.0414x; 1.0414x over previous
"""AFNO (Adaptive Fourier Neural Operator) Trainium2 kernel, 8-core data-parallel.

Per core: one batch element.
  spectral path: rfft2 via packed-real W-DFT + stacked cos/sin H-DFT matmuls,
  block-diag complex MLP (relu, softshrink), irfft2 via inverse DFT matmuls.
  bias path: dense 768x768 per-token linear (Conv1d k=1), staged via DRAM.
Layout rotations use the DMA xbar transpose engine in its native form:
  in [P, k*128] -> out [128, k, P]   (each [P,128] column tile transposed)
with zero-padded 128-minors (pads persist in once-allocated tiles).
All matmul inputs bf16, fp32 PSUM accumulation.
"""

import sys, os
for p in ("/opt/trn_rl_repo", "/root/.axon_site/_ro/trn_rl_repo"):
    if os.path.isdir(p) and p not in sys.path:
        sys.path.insert(0, p)

import numpy as np
import ml_dtypes
from contextlib import ExitStack

import concourse.bass as bass
from concourse import bacc
import concourse.mybir as mybir
import concourse.tile as tile
from concourse.bass import ts
from concourse.bass_utils import run_bass_kernel_spmd

BF16 = mybir.dt.bfloat16
F32 = mybir.dt.float32
NPBF16 = ml_dtypes.bfloat16
AF = mybir.ActivationFunctionType

DIM, H, W, NB = 768, 64, 64, 8
BS = DIM // NB     # 96
LAMBD = 0.01
NCORES = 8
CHUNKS = [(0, 8), (8, 8), (16, 8), (24, 8), (32, 1)]


def _build_consts():
    w = np.arange(W)
    h = np.arange(H)
    rwt = np.zeros((W, W), np.float64)          # lhsT [w, w~] fwd packed W-DFT
    for j in range(33):
        rwt[:, j] = np.cos(2 * np.pi * w * j / 64)
    for t in range(31):
        rwt[:, 33 + t] = -np.sin(2 * np.pi * w * (t + 1) / 64)
    rwt /= 64.0                                 # fwd ortho norm

    ang = 2 * np.pi * np.outer(h, h) / 64
    fha, fhb = np.cos(ang), np.sin(ang)         # A = Ch@P, B = Sh@P
    chi, shi = np.cos(ang), np.sin(ang)         # inverse-H (ifft along h')
    z = np.zeros((H, H))
    chishi = np.concatenate([chi, shi], axis=1)
    nshichi = np.concatenate([-shi, chi], axis=1)
    chishin = np.concatenate([z, chi], axis=1)      # nyq re' -> partitions 64+
    nshichin = np.concatenate([z, -shi], axis=1)

    rwinv = np.zeros((W, W), np.float64)        # [packed64, w] inverse-W
    rwinv[0, :] = 1.0
    for kk in range(1, 32):
        rwinv[kk, :] = 2 * np.cos(2 * np.pi * w * kk / 64)
    for t in range(31):                         # im w'=t+1 at row 32+t
        rwinv[32 + t, :] = -2 * np.sin(2 * np.pi * w * (t + 1) / 64)
    rwinv[63, :] = np.cos(np.pi * w)            # nyq-re at row 63
    rwinv /= 64.0                               # inv ortho norm

    return {n: a.astype(np.float32).astype(NPBF16) for n, a in
            [("rwt", rwt), ("fha", fha), ("fhb", fhb), ("chishi", chishi),
             ("nshichi", nshichi), ("chishin", chishin),
             ("nshichin", nshichin),
             ("rwinv_re", rwinv[0:32]), ("rwinv_im", rwinv[32:64])]}


def build_nc():
    nc = bacc.Bacc("TRN2", target_bir_lowering=False, debug=False)

    xs_d = nc.declare_dram_parameter("xs", [NB, W, BS, H], BF16, isOutput=False)
    xt_d = nc.declare_dram_parameter("xt", [6, 128, 4096], BF16, isOutput=False)
    out_d = nc.declare_dram_parameter("out", [NB, 8, 2, 64, 4, BS], F32,
                                      isOutput=True)

    cds = {}
    for n in ["rwt", "fha", "fhb"]:
        cds[n] = nc.declare_dram_parameter(n, [64, 64], BF16, isOutput=False)
    for n in ["rwinv_re", "rwinv_im"]:
        cds[n] = nc.declare_dram_parameter(n, [32, 64], BF16, isOutput=False)
    for n in ["chishi", "nshichi", "chishin", "nshichin"]:
        cds[n] = nc.declare_dram_parameter(n, [64, 128], BF16, isOutput=False)
    for n in ["w10", "w11", "w11n", "w20", "w21", "w21n"]:
        cds[n] = nc.declare_dram_parameter(n, [BS, NB * BS], BF16, isOutput=False)
    for n in ["b1r", "b1i", "a1r", "a2r", "a1i", "a2i", "bbp"]:
        cds[n] = nc.declare_dram_parameter(n, [BS, NB], F32, isOutput=False)
    wbt_d = nc.declare_dram_parameter("wbt", [6, 128, NB * BS], BF16, isOutput=False)

    with ExitStack() as ctx:
        tc = ctx.enter_context(tile.TileContext(nc))

        cpool = ctx.enter_context(tc.tile_pool(name="consts", bufs=1))
        big = ctx.enter_context(tc.tile_pool(name="big", bufs=1))
        sp = ctx.enter_context(tc.tile_pool(name="spec", bufs=1))
        sm = ctx.enter_context(tc.tile_pool(name="small", bufs=2))
        sm1 = ctx.enter_context(tc.tile_pool(name="small1", bufs=1))
        op = ctx.enter_context(tc.tile_pool(name="outp", bufs=1))
        pp = ctx.enter_context(tc.tile_pool(name="ps", bufs=5, space="PSUM"))
        ppb = ctx.enter_context(tc.tile_pool(name="psb", bufs=2, space="PSUM"))
        dram = ctx.enter_context(tc.tile_pool(name="dram", bufs=1, space="DRAM"))
        bias_d = dram.tile([NB, 8, 2, 64, 4, BS], BF16, tag="bias_scratch",
                           name="bias_scratch")

        ct = {}
        for n in ["rwt", "fha", "fhb"]:
            ct[n] = cpool.tile([64, 64], BF16, tag=f"c_{n}", name=f"c_{n}")
            nc.sync.dma_start(ct[n][:], cds[n][:])
        ct["rwinv_re"] = cpool.tile([32, 64], BF16, tag="c_rwinv_re",
                                    name="c_rwinv_re")
        nc.sync.dma_start(ct["rwinv_re"][:], cds["rwinv_re"][:])
        _rwim = cpool.tile([64, 64], BF16, tag="c_rwinv_im", name="c_rwinv_im")
        nc.sync.dma_start(_rwim[32:64, :], cds["rwinv_im"][:])
        ct["rwinv_im"] = _rwim[32:64, :]
        for n in ["chishi", "nshichi", "chishin", "nshichin"]:
            ct[n] = cpool.tile([64, 128], BF16, tag=f"c_{n}", name=f"c_{n}")
            nc.sync.dma_start(ct[n][:], cds[n][:])
        for n in ["w10", "w11", "w11n", "w20", "w21", "w21n"]:
            ct[n] = cpool.tile([BS, NB * BS], BF16, tag=f"c_{n}", name=f"c_{n}")
            nc.sync.dma_start(ct[n][:], cds[n][:])
        for n in ["b1r", "b1i", "a1r", "a2r", "a1i", "a2i", "bbp"]:
            ct[n] = cpool.tile([BS, NB], F32, tag=f"c_{n}", name=f"c_{n}")
            nc.sync.dma_start(ct[n][:], cds[n][:])
        wbt_t = []
        for kk in range(6):
            t = cpool.tile([128, NB * BS], BF16, tag=f"c_wbt{kk}",
                           name=f"c_wbt{kk}")
            nc.sync.dma_start(t[:], wbt_d[kk])
            wbt_t.append(t)

        # 12KB-class tags; bias phase borrows some for xt chunks
        BIGTAGS = ["xsj", "p1t", "s2a", "s2b", "t6a", "t6b"]

        def ev(i):
            return nc.vector.tensor_copy if (i % 2 == 0) else nc.scalar.copy

        # ========== phase A: bias path -> DRAM scratch ==========
        xtt = []
        for kk in range(6):
            t = big.tile([128, 4096], BF16, tag=BIGTAGS[kk], name=f"xt{kk}")
            nc.sync.dma_start(t[:], xt_d[kk])
            xtt.append(t)
        for j in range(NB):
            for t in range(8):
                pb = ppb.tile([BS, 512], F32, tag="psb")
                for kk in range(6):
                    nc.tensor.matmul(pb[:], wbt_t[kk][:, ts(j, BS)],
                                     xtt[kk][:, ts(t, 512)],
                                     start=(kk == 0), stop=(kk == 5))
                bx = sm.tile([BS, 512], BF16, tag="bx")
                nc.scalar.activation(bx[:], pb[:], AF.Identity,
                                     bias=ct["bbp"][:, j:j + 1])
                # native xbar: [96, 4*128] -> [128=(h2,w), 4, 96]
                bxt = sm.tile([128, 4, BS], BF16, tag="bxt")
                nc.sync.dma_start_transpose(bxt[:], bx[:])
                nc.scalar.dma_start(bias_d[j, t, 0], bxt[0:64])
                nc.scalar.dma_start(bias_d[j, t, 1], bxt[64:128])

        # ========== phase B: spectral path per channel block ==========
        # persistent padded tiles (pads zeroed once)
        p1 = big.tile([W, BS, 128], BF16, tag="p1", name="p1g")
        mixr = sp.tile([H, 33, 128], BF16, tag="mixr", name="mixrg")
        mixi = sp.tile([H, 33, 128], BF16, tag="mixi", name="mixig")
        s5r = sp.tile([BS, 33, 128], BF16, tag="s5r", name="s5rg")
        s5i = sp.tile([BS, 33, 128], BF16, tag="s5i", name="s5ig")
        s6buf = sp.tile([128, BS, 128], BF16, tag="s6buf", name="s6bufg")
        nc.gpsimd.memset(p1[:, :, 64:128], 0.0)
        nc.gpsimd.memset(mixr[:, :, 96:128], 0.0)
        nc.gpsimd.memset(mixi[:, :, 96:128], 0.0)
        nc.gpsimd.memset(s5r[:, :, 64:128], 0.0)
        nc.gpsimd.memset(s5i[:, :, 64:128], 0.0)
        nc.gpsimd.memset(s6buf[0:64, :, 32:128], 0.0)
        nc.gpsimd.memset(s6buf[64:128, :, 0:32], 0.0)
        nc.gpsimd.memset(s6buf[64:128, :, 64:128], 0.0)

        for j in range(NB):
            # S1: packed-real W-DFT -> p1 [w~, (c, hpad128)]
            xsj = big.tile([W, BS, H], BF16, tag="xsj")
            nc.sync.dma_start(xsj[:], xs_d[j])
            xsf = xsj.rearrange("p a b -> p (a b)")
            for t in range(12):
                psx = pp.tile([64, 512], F32, tag="ps")
                nc.tensor.matmul(psx[:], ct["rwt"][:], xsf[:, ts(t, 512)],
                                 start=True, stop=True)
                ev(t)(p1[:, 8 * t:8 * t + 8, 0:64],
                      psx.rearrange("p (a b) -> p a b", a=8))

            # R1 native xbar: [w~, (c x 128)] -> p1t [128=(h|pad), c, w~]
            p1t = big.tile([128, BS, W], BF16, tag="p1t")
            nc.sync.dma_start_transpose(p1t[:], p1.rearrange("p a b -> p (a b)"))

            # S2: H-DFT (A = Ch@P, B = Sh@P) -> [h', (c, w~)]
            s2a = big.tile([H, BS, W], BF16, tag="s2a")
            s2b = big.tile([H, BS, W], BF16, tag="s2b")
            p1tf = p1t[0:64].rearrange("p a b -> p (a b)")
            s2af = s2a.rearrange("p a b -> p (a b)")
            s2bf = s2b.rearrange("p a b -> p (a b)")
            for t in range(12):
                psa = pp.tile([64, 512], F32, tag="ps")
                psb2 = pp.tile([64, 512], F32, tag="ps")
                nc.tensor.matmul(psa[:], ct["fha"][:], p1tf[:, ts(t, 512)],
                                 start=True, stop=True)
                nc.tensor.matmul(psb2[:], ct["fhb"][:], p1tf[:, ts(t, 512)],
                                 start=True, stop=True)
                ev(t)(s2af[:, ts(t, 512)], psa[:])
                ev(t + 1)(s2bf[:, ts(t, 512)], psb2[:])

            # MIX: A/B -> re/im  [h', (w' 33, cpad128)]
            nc.vector.tensor_add(
                mixr[:, 1:32, 0:96],
                s2a[:, :, 1:32].rearrange("p c w -> p w c"),
                s2b[:, :, 33:64].rearrange("p c w -> p w c"))
            nc.gpsimd.tensor_sub(
                mixi[:, 1:32, 0:96],
                s2a[:, :, 33:64].rearrange("p c w -> p w c"),
                s2b[:, :, 1:32].rearrange("p c w -> p w c"))
            nc.vector.tensor_copy(mixr[:, 0, 0:96], s2a[:, :, 0])
            nc.vector.tensor_copy(mixr[:, 32, 0:96], s2a[:, :, 32])
            nc.scalar.mul(mixi[:, 0, 0:96], s2b[:, :, 0], -1.0)
            nc.scalar.mul(mixi[:, 32, 0:96], s2b[:, :, 32], -1.0)

            # rot native: [h', (33 x 128)] -> [128=(c|pad), 33, h']
            mlpr = sp.tile([128, 33, H], BF16, tag="mlpr")
            mlpi = sp.tile([128, 33, H], BF16, tag="mlpi")
            nc.sync.dma_start_transpose(
                mlpr[:], mixr.rearrange("p a b -> p (a b)"))
            nc.sync.dma_start_transpose(
                mlpi[:], mixi.rearrange("p a b -> p (a b)"))

            # MLP layer 1
            r1b = sp.tile([BS, 33, H], BF16, tag="r1b")
            i1b = sp.tile([BS, 33, H], BF16, tag="i1b")
            for (w0, wc) in CHUNKS:
                n = wc * H
                rr = mlpr[0:96, w0:w0 + wc, :]
                ri = mlpi[0:96, w0:w0 + wc, :]
                pr = pp.tile([BS, 512], F32, tag="ps")
                pi = pp.tile([BS, 512], F32, tag="ps")
                nc.tensor.matmul(pr[:, :n], ct["w10"][:, ts(j, BS)], rr,
                                 start=True, stop=False)
                nc.tensor.matmul(pr[:, :n], ct["w11n"][:, ts(j, BS)], ri,
                                 start=False, stop=True)
                nc.tensor.matmul(pi[:, :n], ct["w11"][:, ts(j, BS)], rr,
                                 start=True, stop=False)
                nc.tensor.matmul(pi[:, :n], ct["w10"][:, ts(j, BS)], ri,
                                 start=False, stop=True)
                nc.scalar.activation(r1b[:, w0:w0 + wc, :], pr[:, :n],
                                     AF.Relu, bias=ct["b1r"][:, j:j + 1])
                nc.scalar.activation(i1b[:, w0:w0 + wc, :], pi[:, :n],
                                     AF.Relu, bias=ct["b1i"][:, j:j + 1])

            # MLP layer 2 + softshrink -> s5r/s5i [c, (w' 33, hpad128)]
            for (w0, wc) in CHUNKS:
                n = wc * H
                rr, ri = r1b[:, w0:w0 + wc, :], i1b[:, w0:w0 + wc, :]
                pr = pp.tile([BS, 512], F32, tag="ps")
                pi = pp.tile([BS, 512], F32, tag="ps")
                nc.tensor.matmul(pr[:, :n], ct["w20"][:, ts(j, BS)], rr,
                                 start=True, stop=False)
                nc.tensor.matmul(pr[:, :n], ct["w21n"][:, ts(j, BS)], ri,
                                 start=False, stop=True)
                nc.tensor.matmul(pi[:, :n], ct["w21"][:, ts(j, BS)], rr,
                                 start=True, stop=False)
                nc.tensor.matmul(pi[:, :n], ct["w20"][:, ts(j, BS)], ri,
                                 start=False, stop=True)
                sar = sm1.tile([BS, wc, H], BF16, tag="sar")
                sbr = sm1.tile([BS, wc, H], BF16, tag="sbr")
                sai = sm1.tile([BS, wc, H], BF16, tag="sai")
                sbi = sm1.tile([BS, wc, H], BF16, tag="sbi")
                nc.scalar.activation(sar[:], pr[:, :n], AF.Relu,
                                     bias=ct["a1r"][:, j:j + 1])
                nc.scalar.activation(sbr[:], pr[:, :n], AF.Relu,
                                     bias=ct["a2r"][:, j:j + 1], scale=-1.0)
                nc.scalar.activation(sai[:], pi[:, :n], AF.Relu,
                                     bias=ct["a1i"][:, j:j + 1])
                nc.scalar.activation(sbi[:], pi[:, :n], AF.Relu,
                                     bias=ct["a2i"][:, j:j + 1], scale=-1.0)
                nc.vector.tensor_sub(s5r[:, w0:w0 + wc, 0:64], sar[:], sbr[:])
                nc.gpsimd.tensor_sub(s5i[:, w0:w0 + wc, 0:64], sai[:], sbi[:])

            # rot native: [c, (33 x 128)] -> [128=(h'|pad), 33, c]
            t5r = sp.tile([128, 33, BS], BF16, tag="mlpr")
            t5i = sp.tile([128, 33, BS], BF16, tag="mlpi")
            nc.sync.dma_start_transpose(
                t5r[:], s5r.rearrange("p a b -> p (a b)"))
            nc.sync.dma_start_transpose(
                t5i[:], s5i.rearrange("p a b -> p (a b)"))

            # S4 inverse-H -> s6buf [128=(reh|imh), (c, minor128)]
            for ci in range(8):
                w0 = ci * 4
                rr = t5r[0:64, w0:w0 + 4, :]
                ri = t5i[0:64, w0:w0 + 4, :]
                p4 = pp.tile([128, 384], F32, tag="ps")
                nc.tensor.matmul(p4[:], ct["chishi"][:], rr,
                                 start=True, stop=False)
                nc.tensor.matmul(p4[:], ct["nshichi"][:], ri,
                                 start=False, stop=True)
                p43 = p4.rearrange("p (a b) -> p a b", a=4)
                # re half -> minor slots w0..w0+3 ; im half -> 32 + (w'-1)
                ev(ci)(s6buf[0:64, :, w0:w0 + 4],
                       p43[0:64].rearrange("p a b -> p b a"))
                if w0 == 0:   # im w'=0 (DC) discarded by irfft
                    ev(ci + 1)(s6buf[64:128, :, 32:35],
                               p43[64:128, 1:4].rearrange("p a b -> p b a"))
                else:
                    ev(ci + 1)(s6buf[64:128, :, 32 + w0 - 1:32 + w0 + 3],
                               p43[64:128].rearrange("p a b -> p b a"))
            p4n = pp.tile([128, BS], F32, tag="ps")
            nc.tensor.matmul(p4n[:], ct["chishin"][:], t5r[0:64, 32, :],
                             start=True, stop=False)
            nc.tensor.matmul(p4n[:], ct["nshichin"][:], t5i[0:64, 32, :],
                             start=False, stop=True)
            nc.vector.tensor_copy(s6buf[64:128, :, 63], p4n[64:128, :])

            # xbar native: re -> t6a rows 0..31, im+nyq -> t6b rows 32..63
            t6a = big.tile([128, BS, 64], BF16, tag="t6a")
            t6b = big.tile([128, BS, 64], BF16, tag="t6b")
            nc.sync.dma_start_transpose(
                t6a[:], s6buf[0:64].rearrange("p a b -> p (a b)"))
            nc.sync.dma_start_transpose(
                t6b[:], s6buf[64:128].rearrange("p a b -> p (a b)"))

            # S5 inverse-W (two K=32 matmuls) -> spatial [w, (c, h)]
            spo = big.tile([W, BS, H], BF16, tag="s2b", name=f"spo{j}")
            spof = spo.rearrange("p a b -> p (a b)")
            t6af = t6a[0:32].rearrange("p a b -> p (a b)")
            t6bf = t6b[32:64].rearrange("p a b -> p (a b)")
            for t in range(12):
                p5 = pp.tile([64, 512], F32, tag="ps")
                nc.tensor.matmul(p5[:], ct["rwinv_re"][:], t6af[:, ts(t, 512)],
                                 start=True, stop=False)
                nc.tensor.matmul(p5[:], ct["rwinv_im"], t6bf[:, ts(t, 512)],
                                 start=False, stop=True)
                ev(t)(spof[:, ts(t, 512)], p5[:])

            # final add with bias tiles from DRAM (contiguous layouts)
            for t in range(8):
                bt1 = sm.tile([64, 4, BS], BF16, tag="bt1")
                bt2 = sm.tile([64, 4, BS], BF16, tag="bt2")
                nc.scalar.dma_start(bt1[:], bias_d[j, t, 0])
                nc.scalar.dma_start(bt2[:], bias_d[j, t, 1])
                oadd1 = op.tile([64, 4, BS], F32, tag="oadd1")
                oadd2 = op.tile([64, 4, BS], F32, tag="oadd2")
                a1 = nc.vector.tensor_add if (t % 2 == 0) else \
                    nc.gpsimd.tensor_add
                a2 = nc.gpsimd.tensor_add if (t % 2 == 0) else \
                    nc.vector.tensor_add
                a1(oadd1[:], bt1[:],
                   spo[:, :, 8 * t:8 * t + 8:2].rearrange("p c h -> p h c"))
                a2(oadd2[:], bt2[:],
                   spo[:, :, 8 * t + 1:8 * t + 8:2].rearrange("p c h -> p h c"))
                nc.sync.dma_start(out_d[j, t, 0], oadd1[:])
                nc.sync.dma_start(out_d[j, t, 1], oadd2[:])

    return nc


_nc_cache = None


def _get_nc():
    global _nc_cache
    if _nc_cache is None:
        _nc_cache = build_nc()
        _nc_cache.finalize()
    return _nc_cache


def make_in_maps(x, w1, b1, w2, b2, Wb, bb):
    shared = dict(_build_consts())
    shared["w10"] = np.concatenate([w1[0][b] for b in range(NB)], 1).astype(NPBF16)
    shared["w11"] = np.concatenate([w1[1][b] for b in range(NB)], 1).astype(NPBF16)
    shared["w11n"] = np.concatenate([-w1[1][b] for b in range(NB)], 1).astype(NPBF16)
    shared["w20"] = np.concatenate([w2[0][b] for b in range(NB)], 1).astype(NPBF16)
    shared["w21"] = np.concatenate([w2[1][b] for b in range(NB)], 1).astype(NPBF16)
    shared["w21n"] = np.concatenate([-w2[1][b] for b in range(NB)], 1).astype(NPBF16)
    shared["b1r"] = np.ascontiguousarray(b1[0].T).astype(np.float32)
    shared["b1i"] = np.ascontiguousarray(b1[1].T).astype(np.float32)
    shared["a1r"] = np.ascontiguousarray((b2[0] - LAMBD).T).astype(np.float32)
    shared["a2r"] = np.ascontiguousarray((-b2[0] - LAMBD).T).astype(np.float32)
    shared["a1i"] = np.ascontiguousarray((b2[1] - LAMBD).T).astype(np.float32)
    shared["a2i"] = np.ascontiguousarray((-b2[1] - LAMBD).T).astype(np.float32)
    shared["bbp"] = np.ascontiguousarray(bb.reshape(NB, BS).T).astype(np.float32)
    shared["wbt"] = np.ascontiguousarray(Wb.T.reshape(6, 128, DIM)).astype(NPBF16)

    in_maps = []
    for b in range(NCORES):
        m = dict(shared)
        xg = x[b].reshape(H, W, DIM)
        m["xs"] = np.ascontiguousarray(
            xg.reshape(H, W, NB, BS).transpose(2, 1, 3, 0)).astype(NPBF16)
        m["xt"] = np.ascontiguousarray(x[b].T.reshape(6, 128, 4096)).astype(NPBF16)
        in_maps.append(m)
    return in_maps


def kernel(x, w1, b1, w2, b2, Wb, bb, _trace=False):
    nc = _get_nc()
    in_maps = make_in_maps(np.asarray(x, np.float32), np.asarray(w1, np.float32),
                           np.asarray(b1, np.float32), np.asarray(w2, np.float32),
                           np.asarray(b2, np.float32), np.asarray(Wb, np.float32),
                           np.asarray(bb, np.float32))
    res = run_bass_kernel_spmd(nc, in_maps, list(range(NCORES)), trace=_trace)
    outs = []
    for b in range(NCORES):
        o = np.asarray(res.results[b]["out"], np.float32)
        # [j, t, (hp, w), q, c] ; h = 8t + 2q + hp, c_full = 96j + c
        o = o.reshape(NB, 8, 2, W, 4, BS).transpose(1, 4, 2, 3, 0, 5)
        outs.append(o.reshape(H * W, DIM))
    full = np.stack(outs, axis=0)
    if _trace:
        return full, res
    return full
